# revision 1
# baseline (speedup 1.0000x reference)
"""Trainium2 Bass kernel for batched differentiable mean-variance optimization.

Problem: for each of 256 samples, solve
    min 0.5 y^T Sigma y  s.t.  mu^T y = 1, y >= 0
via 150 unrolled projected-gradient iterations (step = 1/lambda_max via 20
power iterations), then normalize to portfolio weights.

Strategy (per core, 32 samples, pure data parallel across 8 cores):
- Two resident passes of 16 samples (fp32 Sigma tiles live in SBUF: 16 MB).
- Matvec Sigma @ y as out = y^T Sigma (Sigma symmetric): y chunks [128,1] are
  the PE stationary operand, Sigma row-chunks [128,512] stream as the moving
  operand. 4 samples run concurrently in distinct 32-column groups of the PE
  array (tile_position), quadrupling effective stream bandwidth.
- Projection onto {y>=0, mu@y=1} replaces the reference's 50-step bisection
  with K Newton/active-set iterations (identical root to fp32 precision),
  warm-started across PGD steps. All elementwise work runs on DVE in a dense
  [32, 128] layout: partition = 8*quarter + sample, free = element-in-quarter.
  Masked sums fuse into single scalar_tensor_tensor ops with accum_out;
  cross-partition sums + per-partition broadcast happen in one small PE
  matmul against a block-replicated G8 matrix.
- Power iteration runs unnormalized (scale-invariant) + one Rayleigh quotient.
- A small DMA repack moves matvec PSUM rows into the projection layout.
"""

import os
import numpy as np
from contextlib import ExitStack

N = 512
NCORES = 8
SPC = 32          # samples per core
PASS_N = 16       # resident samples per pass
SG = 8            # samples per subgroup (2 subgroups pipeline per pass)
POWER_ITERS = 20
PGD_ITERS = 150
NEWTON_K = 6

_PROGRAM_CACHE = {}


def _build_program(spc=SPC, pass_n=PASS_N, sg_n=SG, power_iters=POWER_ITERS,
                   pgd_iters=PGD_ITERS, newton_k=NEWTON_K):
    import concourse.bacc as bacc
    import concourse.tile as tile
    from concourse import mybir

    Alu = mybir.AluOpType
    F32 = mybir.dt.float32
    F32R = mybir.dt.float32r
    use_f32r = os.environ.get("KM_F32R", "1") == "1"
    MMDT = F32R if use_f32r else F32
    global SPC, PASS_N, SG, POWER_ITERS, PGD_ITERS, NEWTON_K
    SPC, PASS_N, SG = spc, pass_n, sg_n
    POWER_ITERS, PGD_ITERS, NEWTON_K = power_iters, pgd_iters, newton_k

    nc = bacc.Bacc(
        "TRN2",
        target_bir_lowering=False,
        debug=False,
        enable_asserts=False,
        num_devices=NCORES,
    )

    mu_dram = nc.dram_tensor("mu_in", [SPC, N], F32, kind="ExternalInput").ap()
    sig_dram = nc.dram_tensor("sigma_in", [SPC, N, N], F32, kind="ExternalInput").ap()
    P_ = 4 * SG
    g8_dram = nc.dram_tensor("g8_in", [P_, P_], F32, kind="ExternalInput").ap()
    id_dram = nc.dram_tensor("ident_in", [P_, P_], F32, kind="ExternalInput").ap()
    w_dram = nc.dram_tensor("w_out", [SPC, N], F32, kind="ExternalOutput").ap()

    with tile.TileContext(nc) as tc, ExitStack() as ctx:
        const_pool = ctx.enter_context(tc.tile_pool(name="const", bufs=1))
        sig_pool = ctx.enter_context(tc.tile_pool(name="sig", bufs=1))
        state_pool = ctx.enter_context(tc.tile_pool(name="state", bufs=1))
        adma_pool = ctx.enter_context(tc.tile_pool(name="adma", bufs=3))
        # PSUM budget is 8 banks: 4 matvec tiles (one per subgroup x group),
        # 2 transpose tiles, 2 newton tiles -- all bufs=1, tags per subgroup.
        mv_pool = ctx.enter_context(tc.tile_pool(name="mv", bufs=1, space="PSUM"))
        tr_pool = ctx.enter_context(tc.tile_pool(name="tr", bufs=1, space="PSUM"))
        nw_pool = ctx.enter_context(tc.tile_pool(name="nw", bufs=1, space="PSUM"))

        g8_sb = const_pool.tile([P_, P_], F32)
        nc.sync.dma_start(out=g8_sb, in_=g8_dram)
        id_sb = const_pool.tile([P_, P_], F32)
        nc.sync.dma_start(out=id_sb, in_=id_dram)

        def emit_pass(s0):
            """Process samples [s0, s0+PASS_N)."""
            # Sigma resident: [part p, sample, chunk c, elem] = Sigma[s][128c+p, e]
            # For fp32r matmuls the producer must round: DMA to an fp32
            # staging tile, then a DVE copy converts into the resident tile.
            sig_sb = sig_pool.tile([128, PASS_N, 4, N], MMDT, tag="sig")
            for b in range(PASS_N):
                if use_f32r:
                    sstage = adma_pool.tile([128, 4, N], F32, tag="sigstage", bufs=2)
                    nc.sync.dma_start(
                        out=sstage,
                        in_=sig_dram[s0 + b].rearrange("(c p) e -> p c e", p=128),
                    )
                    nc.vector.tensor_copy(sig_sb[:, b], sstage)
                else:
                    nc.sync.dma_start(
                        out=sig_sb[:, b],
                        in_=sig_dram[s0 + b].rearrange("(c p) e -> p c e", p=128),
                    )

            for sg in range(PASS_N // SG):
                emit_subgroup(s0, sg, sig_sb)

        def emit_subgroup(s0, sg, sig_sb):
            """Samples [s0+sg*SG, s0+(sg+1)*SG). A4 layout: [32,128] tiles;
            fp32r path: partition pi = 4b + q (sample-major, so every DMA
            write is partition-contiguous); fp32 path: pi = 8q + b. Free f:
            element 128q+f of sample b. x_B is the B layout: partition =
            element within quarter, free dims (b, q) resp. (q, b)."""
            tg = f"sg{sg}"
            P = SG * 4  # partitions used in A4 tiles (=32)

            mu_rep = state_pool.tile([P, 128], F32, tag=f"{tg}_mur")
            if use_f32r:
                for b in range(SG):
                    nc.sync.dma_start(
                        out=mu_rep[4 * b : 4 * b + 4, :],
                        in_=mu_dram[s0 + sg * SG + b : s0 + sg * SG + b + 1, :],
                    )
            else:
                for q in range(4):
                    nc.sync.dma_start(
                        out=mu_rep[SG * q : SG * (q + 1), :],
                        in_=mu_dram[s0 + sg * SG : s0 + (sg + 1) * SG,
                                    128 * q : 128 * (q + 1)],
                    )
            invmu = state_pool.tile([P, 128], F32, tag=f"{tg}_imu")
            nc.vector.reciprocal(invmu, mu_rep)
            musq = state_pool.tile([P, 128], F32, tag=f"{tg}_msq")
            nc.vector.tensor_mul(musq, mu_rep, mu_rep)

            xb_dims = [128, SG, 4] if use_f32r else [128, 4, SG]
            x_B = state_pool.tile(xb_dims, MMDT, tag=f"{tg}_xB")
            ys_A4 = state_pool.tile([P, 128], F32, tag=f"{tg}_ys")
            u_t = state_pool.tile([P, 128], F32, tag=f"{tg}_u")
            r_t = state_pool.tile([P, 128], F32, tag=f"{tg}_r")
            muv = state_pool.tile([P, 128], F32, tag=f"{tg}_muv")
            t_t = state_pool.tile([P, 128], F32, tag=f"{tg}_t")
            prod = state_pool.tile([P, 2, 128], F32, tag=f"{tg}_prod")
            ab = state_pool.tile([P, 2], F32, tag=f"{tg}_ab")
            neglam = state_pool.tile([P, 1], F32, tag=f"{tg}_nl")
            lam = state_pool.tile([P, 1], F32, tag=f"{tg}_lam")
            rb = state_pool.tile([P, 1], F32, tag=f"{tg}_rb")
            bmax = state_pool.tile([P, 1], F32, tag=f"{tg}_bm")
            negstep = state_pool.tile([P, 1], F32, tag=f"{tg}_ns")
            invnegstep = state_pool.tile([P, 1], F32, tag=f"{tg}_ins")
            nd = state_pool.tile([P, 2], F32, tag=f"{tg}_nd")

            n_groups = (SG + 3) // 4

            def matvec_repack(dst):
                """x_B (B layout) -> Sigma@x -> A4 [32,128] tile `dst`.

                fp32r path: per-sample 4-chunk accumulation chain into a
                [1, N] psum bank (full-column mode; fp32r forbids col-group
                tiling), idle ScalarE stages to SBUF, one DMA scatters to
                the A4 layout (partition 8q+b).
                fp32 path: 4 samples run in distinct 32-column groups of the
                PE array, outputs at partitions 32j of a shared bank."""
                if use_f32r:
                    stage = adma_pool.tile([1, SG, N], F32, tag=f"{tg}_st", bufs=1)
                    for b in range(SG):
                        s_loc = sg * SG + b
                        ps = mv_pool.tile([1, N], F32, tag=f"{tg}_mv{b % 2}",
                                          name=f"mv_{tg}_{b % 2}")
                        for p in range(4):
                            nc.tensor.matmul(
                                ps[0:1, :],
                                x_B[:, b, p : p + 1],
                                sig_sb[:, s_loc, p, :],
                                start=(p == 0),
                                stop=(p == 3),
                            )
                        nc.scalar.copy(stage[0:1, b, :], ps[0:1, :])
                        nc.sync.dma_start(
                            out=dst[4 * b : 4 * b + 4, :],
                            in_=stage[0:1, b, :],
                        )
                    return
                psums = []
                for g in range(n_groups):
                    ps = mv_pool.tile([128, N], F32, tag=f"{tg}_mv{g}")
                    if os.environ.get("KM_SIM_SAFE"):
                        nc.vector.memset(ps, 0.0)
                    for p in range(4):
                        for j in range(min(4, SG - 4 * g)):
                            b = 4 * g + j
                            s_loc = sg * SG + b
                            nc.tensor.matmul(
                                ps[32 * j : 32 * j + 1, :],
                                x_B[:, p, b : b + 1],
                                sig_sb[:, s_loc, p, :],
                                start=(p == 0),
                                stop=(p == 3),
                                tile_position=(0, 32 * j),
                            )
                    psums.append(ps)
                for g in range(n_groups):
                    ps = psums[g]
                    nj = min(4, SG - 4 * g)
                    stage = adma_pool.tile([128, N], F32, tag=f"{tg}_st{g}")
                    # Compute engines cannot stride partitions; copy the whole
                    # contiguous range (cost is free-dim-bound anyway).
                    np_ = 32 * (nj - 1) + 1
                    nc.scalar.copy(stage[0:np_, :], ps[0:np_, :])
                    for q in range(4):
                        nc.sync.dma_start(
                            out=dst[SG * q + 4 * g : SG * q + 4 * g + nj, :],
                            in_=stage[0 : 32 * nj : 32, 128 * q : 128 * (q + 1)],
                        )

            def to_B(src_a4):
                """A4 [32,128] -> x_B via PE transpose + copy."""
                trp = tr_pool.tile([128, P], F32, tag=f"{tg}_tr")
                nc.tensor.transpose(trp, src_a4, id_sb)
                if use_f32r:
                    nc.vector.tensor_copy(
                        x_B, trp.rearrange("p (b q) -> p b q", q=4))
                else:
                    nc.vector.tensor_copy(
                        x_B, trp.rearrange("p (q b) -> p q b", q=4))

            stop_at = os.environ.get("KM_DBG_STOP", "full")

            def wout(src_t):
                if use_f32r:
                    for b in range(SG):
                        nc.sync.dma_start(
                            out=w_dram[s0 + sg * SG + b : s0 + sg * SG + b + 1, :],
                            in_=src_t[4 * b : 4 * b + 4, :],
                        )
                else:
                    for q in range(4):
                        nc.sync.dma_start(
                            out=w_dram[s0 + sg * SG : s0 + (sg + 1) * SG,
                                       128 * q : 128 * (q + 1)],
                            in_=src_t[SG * q : SG * (q + 1), :],
                        )

            def gmm(rhs_sb, out_ps, n):
                """Cross-partition sum + broadcast: out[pi, i] = sum over same-
                sample partitions of rhs[:, i]."""
                nc.tensor.matmul(
                    out_ps[:, 0:n],
                    g8_sb,
                    rhs_sb[:, 0:n],
                    start=True,
                    stop=True,
                )

            # ---- power iteration (unnormalized) ----
            if use_f32r:
                ones_f = adma_pool.tile(xb_dims, F32, tag=f"{tg}_ones", bufs=1)
                nc.vector.memset(ones_f, 1.0)
                nc.vector.tensor_copy(x_B, ones_f)
            else:
                nc.vector.memset(x_B, 1.0)
            v_a4 = None
            for k in range(POWER_ITERS):
                v_a4 = adma_pool.tile([P, 128], F32, tag=f"{tg}_va4")
                matvec_repack(v_a4)
                to_B(v_a4)
            w_a4 = adma_pool.tile([P, 128], F32, tag=f"{tg}_wa4")
            matvec_repack(w_a4)
            # Rayleigh: lmax = (v.w)/(v.v); negstep = -1/lmax; invnegstep = -lmax
            nc.vector.scalar_tensor_tensor(
                out=prod[:, 0, :], in0=v_a4, scalar=0.0, in1=w_a4,
                op0=Alu.add, op1=Alu.mult, accum_out=nd[:, 0:1],
            )
            nc.vector.scalar_tensor_tensor(
                out=prod[:, 1, :], in0=v_a4, scalar=0.0, in1=v_a4,
                op0=Alu.add, op1=Alu.mult, accum_out=nd[:, 1:2],
            )
            nwp = nw_pool.tile([P, 2], F32, tag=f"{tg}_nw")
            gmm(nd, nwp, 2)
            # num=nwp[:,0] (v.w), den=nwp[:,1] (v.v)
            nc.vector.reciprocal(rb, nwp[:, 0:1])            # 1/(v.w)
            nc.vector.scalar_tensor_tensor(
                out=negstep, in0=nwp[:, 1:2], scalar=-1.0, in1=rb,
                op0=Alu.mult, op1=Alu.mult,
            )                                                # -(v.v)/(v.w) = -1/lmax
            nc.vector.reciprocal(bmax, nwp[:, 1:2])          # 1/(v.v)
            nc.vector.scalar_tensor_tensor(
                out=invnegstep, in0=nwp[:, 0:1], scalar=-1.0, in1=bmax,
                op0=Alu.mult, op1=Alu.mult,
            )                                                # -lmax

            def newton(r_ap, muv_ap):
                for _ in range(NEWTON_K):
                    nc.vector.scalar_tensor_tensor(
                        out=prod[:, 0, :], in0=r_ap, scalar=neglam[:, 0:1],
                        in1=muv_ap, op0=Alu.is_gt, op1=Alu.mult,
                        accum_out=ab[:, 0:1],
                    )
                    nc.vector.scalar_tensor_tensor(
                        out=prod[:, 1, :], in0=r_ap, scalar=neglam[:, 0:1],
                        in1=musq, op0=Alu.is_gt, op1=Alu.mult,
                        accum_out=ab[:, 1:2],
                    )
                    abp = nw_pool.tile([P, 2], F32, tag=f"{tg}_nw")
                    gmm(ab, abp, 2)
                    nc.vector.tensor_scalar(
                        out=bmax, in0=abp[:, 1:2], scalar1=1e-30, scalar2=None,
                        op0=Alu.max,
                    )
                    nc.vector.reciprocal(rb, bmax)
                    nc.vector.scalar_tensor_tensor(
                        out=neglam, in0=abp[:, 0:1], scalar=-1.0, in1=rb,
                        op0=Alu.add, op1=Alu.mult,
                    )

            if stop_at == "power":
                wout(w_a4)
                return

            # ---- y0 = project(ones) ----
            nc.vector.memset(neglam, -1e30)
            newton(invmu, mu_rep)  # u=ones: r=invmu, muv=mu
            nc.vector.tensor_scalar(
                out=lam, in0=neglam, scalar1=-1.0, scalar2=None, op0=Alu.mult
            )
            nc.vector.tensor_scalar(
                out=t_t, in0=mu_rep, scalar1=lam[:, 0:1], scalar2=1.0,
                op0=Alu.mult, op1=Alu.add,
            )
            nc.vector.tensor_scalar(
                out=ys_A4, in0=t_t, scalar1=0.0, scalar2=negstep[:, 0:1],
                op0=Alu.max, op1=Alu.mult,
            )
            to_B(ys_A4)
            if stop_at == "y0":
                wout(ys_A4)
                return

            # ---- PGD ----
            y_fin = None
            for k in range(PGD_ITERS):
                pdma = adma_pool.tile([P, 128], F32, tag=f"{tg}_pd")
                matvec_repack(pdma)  # p' = -step * Sigma y
                # u = y + p' = ys*(-lmax) + p'
                nc.vector.scalar_tensor_tensor(
                    out=u_t, in0=ys_A4, scalar=invnegstep[:, 0:1], in1=pdma,
                    op0=Alu.mult, op1=Alu.add,
                )
                nc.vector.tensor_mul(r_t, u_t, invmu)
                nc.vector.tensor_mul(muv, u_t, mu_rep)
                newton(r_t, muv)
                nc.vector.tensor_scalar(
                    out=lam, in0=neglam, scalar1=-1.0, scalar2=None, op0=Alu.mult
                )
                nc.vector.scalar_tensor_tensor(
                    out=t_t, in0=mu_rep, scalar=lam[:, 0:1], in1=u_t,
                    op0=Alu.mult, op1=Alu.add,
                )
                if k < PGD_ITERS - 1:
                    nc.vector.tensor_scalar(
                        out=ys_A4, in0=t_t, scalar1=0.0, scalar2=negstep[:, 0:1],
                        op0=Alu.max, op1=Alu.mult,
                    )
                    to_B(ys_A4)
                else:
                    y_fin = state_pool.tile([P, 128], F32, tag=f"{tg}_yf")
                    nc.vector.tensor_scalar(
                        out=y_fin, in0=t_t, scalar1=0.0, scalar2=None, op0=Alu.max
                    )

            if stop_at == "pgd":
                wout(y_fin)
                return

            # ---- postprocess ----
            # valid = any(mu > 1e-6) per sample
            cnt = state_pool.tile([P, 1], F32, tag=f"{tg}_cnt")
            nc.vector.tensor_scalar(
                out=prod[:, 0, :], in0=mu_rep, scalar1=1e-6, scalar2=None,
                op0=Alu.is_gt, op1=Alu.add, accum_out=cnt,
            )
            cntp = nw_pool.tile([P, 2], F32, tag=f"{tg}_nw")
            gmm(cnt, cntp, 1)
            mv_ = state_pool.tile([P, 1], F32, tag=f"{tg}_mvd")
            nc.vector.tensor_scalar(
                out=mv_, in0=cntp[:, 0:1], scalar1=0.5, scalar2=None, op0=Alu.is_gt
            )
            omv = state_pool.tile([P, 1], F32, tag=f"{tg}_omv")
            nc.vector.tensor_scalar(
                out=omv, in0=mv_, scalar1=-1.0, scalar2=1.0, op0=Alu.mult, op1=Alu.add
            )
            y2 = state_pool.tile([P, 128], F32, tag=f"{tg}_y2")
            nc.vector.tensor_scalar(
                out=y2, in0=y_fin, scalar1=mv_[:, 0:1], scalar2=omv[:, 0:1],
                op0=Alu.mult, op1=Alu.add,
            )
            # s = sum(y2); w1 = y2*(1/s)*ok + (1-ok)/n ; ok = |s|>1e-6
            sp = state_pool.tile([P, 1], F32, tag=f"{tg}_sp")
            nc.vector.tensor_scalar(
                out=prod[:, 0, :], in0=y2, scalar1=1.0, scalar2=None,
                op0=Alu.mult, op1=Alu.add, accum_out=sp,
            )
            spp = nw_pool.tile([P, 2], F32, tag=f"{tg}_nw")
            gmm(sp, spp, 1)
            ok = state_pool.tile([P, 1], F32, tag=f"{tg}_ok")
            # s >= 0 always (y2 elementwise nonnegative), so |s| == s here.
            nc.vector.tensor_scalar(
                out=ok, in0=spp[:, 0:1], scalar1=1e-6, scalar2=None, op0=Alu.is_gt
            )
            nc.vector.tensor_scalar(
                out=bmax, in0=spp[:, 0:1], scalar1=1e-30, scalar2=None, op0=Alu.max
            )
            nc.vector.reciprocal(rb, bmax)
            sc = state_pool.tile([P, 1], F32, tag=f"{tg}_sc")
            nc.vector.tensor_mul(sc, rb, ok)
            off = state_pool.tile([P, 1], F32, tag=f"{tg}_off")
            nc.vector.tensor_scalar(
                out=off, in0=ok, scalar1=-1.0 / N, scalar2=1.0 / N,
                op0=Alu.mult, op1=Alu.add,
            )
            w1 = state_pool.tile([P, 128], F32, tag=f"{tg}_w1")
            nc.vector.tensor_scalar(
                out=w1, in0=y2, scalar1=sc[:, 0:1], scalar2=off[:, 0:1],
                op0=Alu.mult, op1=Alu.add,
            )
            # renormalize
            s2 = state_pool.tile([P, 1], F32, tag=f"{tg}_s2")
            nc.vector.tensor_scalar(
                out=prod[:, 0, :], in0=w1, scalar1=1.0, scalar2=None,
                op0=Alu.mult, op1=Alu.add, accum_out=s2,
            )
            s2p = nw_pool.tile([P, 2], F32, tag=f"{tg}_nw")
            gmm(s2, s2p, 1)
            nc.vector.reciprocal(rb, s2p[:, 0:1])
            wf = state_pool.tile([P, 128], F32, tag=f"{tg}_wf")
            nc.vector.tensor_scalar(
                out=wf, in0=w1, scalar1=rb[:, 0:1], scalar2=None, op0=Alu.mult
            )
            wout(wf)

        for s0 in range(0, SPC, PASS_N):
            emit_pass(s0)

    nc.compile()
    return nc


def _get_program():
    if "nc" not in _PROGRAM_CACHE:
        _PROGRAM_CACHE["nc"] = _build_program()
    return _PROGRAM_CACHE["nc"]


def kernel(predicted_returns: np.ndarray, covariance_matrix: np.ndarray) -> np.ndarray:
    from concourse.bass_utils import run_bass_kernel_spmd

    mu = np.ascontiguousarray(predicted_returns, dtype=np.float32)
    sig = np.ascontiguousarray(covariance_matrix, dtype=np.float32)
    batch = mu.shape[0]
    assert batch == NCORES * SPC and mu.shape[1] == N

    if os.environ.get("KM_F32R", "1") == "1":
        g8 = np.kron(np.eye(SG, dtype=np.float32), np.ones((4, 4), np.float32))
    else:
        g8 = np.tile(np.eye(SG, dtype=np.float32), (4, 4))
    ident = np.eye(4 * SG, dtype=np.float32)

    nc = _get_program()
    in_maps = []
    for c in range(NCORES):
        sl = slice(c * SPC, (c + 1) * SPC)
        in_maps.append(
            {
                "mu_in": mu[sl],
                "sigma_in": sig[sl],
                "g8_in": g8,
                "ident_in": ident,
            }
        )
    res = run_bass_kernel_spmd(nc, in_maps, core_ids=list(range(NCORES)))
    out = np.concatenate([r["w_out"] for r in res.results], axis=0)
    return out.astype(np.float32)


if __name__ == "__main__":
    rng = np.random.default_rng(0)
    mu = (0.05 + 0.1 * rng.random((NCORES * SPC, N))).astype(np.float32)
    A = rng.standard_normal((4, N, N)).astype(np.float32)
    sig = np.einsum("bik,bjk->bij", A, A) / N + 0.1 * np.eye(N, dtype=np.float32)
    sig = np.tile(sig, (64, 1, 1)).astype(np.float32)
    w = kernel(mu, sig)
    print(w.shape, w.sum(axis=1)[:4])



# revision 3
# speedup vs baseline: 1.2398x; 1.2398x over previous
"""Trainium2 Bass kernel for batched differentiable mean-variance optimization.

Problem: for each of 256 samples, solve
    min 0.5 y^T Sigma y  s.t.  mu^T y = 1, y >= 0
via 150 unrolled projected-gradient iterations (step = 1/lambda_max via power
iteration), then normalize to portfolio weights.  Pure data parallel across 8
cores (32 samples/core).

Design (v2):
- Sigma is shipped from the host as a precision pair: S1 = bf16(Sigma)
  (16 MB/core) plus S2 = fp8_e4m3((Sigma - S1) * 4096) (8 MB/core).  Both stay
  SBUF-resident for all 32 samples.
- Matvec Sigma @ y streams S1 chunks as the PE moving operand at 1 col/cycle
  with 4 samples running concurrently in distinct 32-column groups
  (tile_position), i.e. 512 cycles/sample/matvec.  Error-decay analysis shows
  bf16 matvec noise injected at PGD iter k fades by ~0.976^(150-k), so only
  the last 40 iterations add the fp8-scaled S2 correction matmuls
  (compensated product, ~fp32-quality fixed point) at 2x PE cost.
- Power iteration runs 10 unnormalized steps + Rayleigh quotient (step size
  only needs ~1e-3 accuracy).
- Projection onto {y>=0, mu@y=1}: warm-started Newton/active-set iterations
  (K=3/iter) in a dense A4 layout [64, 128]: partition = 4*sample+quarter.
  Masked sums fuse into scalar_tensor_tensor with accum_out; the 4-partition
  per-sample reduce+broadcast is one tiny PE matmul against a block G matrix.
  ScalarE (ACT) takes the PSUM drains, the relu projection, and the
  PSUM->SBUF transpose copies so DVE keeps only the tensor_tensor work.
- Two subgroups of 16 samples self-pipeline via the Tile list scheduler
  (PE matvec of one subgroup overlaps DVE projection of the other).
- Final weights w = y/sum(y) are computed from ys = -step*mu*z directly
  (per-sample positive rescale cancels in the normalization; relu/renorm
  of the reference are no-ops here since w >= 0 elementwise).
"""

import os
import numpy as np
from contextlib import ExitStack

N = 512
NCORES = 8
SPC = 32            # samples per core (all SBUF-resident)
SGN = 16            # samples per subgroup
POWER_ITERS = 10
PGD_ITERS = 150
SPLIT_FROM = 110    # first PGD iter that applies the S2 correction
NEWTON_K = 3        # warm-started Newton iters per projection
NEWTON_K0 = 8       # cold-start Newton iters for y0
S2_SCALE = 4096.0

_PROGRAM_CACHE = {}


def _build_program():
    import concourse.bacc as bacc
    import concourse.tile as tile
    from concourse import mybir

    Alu = mybir.AluOpType
    Act = mybir.ActivationFunctionType
    F32 = mybir.dt.float32
    BF16 = mybir.dt.bfloat16
    F8 = mybir.dt.float8e4

    nc = bacc.Bacc(
        "TRN2",
        target_bir_lowering=False,
        debug=False,
        enable_asserts=False,
        num_devices=NCORES,
    )

    mu_dram = nc.dram_tensor("mu_in", [SPC, N], F32, kind="ExternalInput").ap()
    s1_dram = nc.dram_tensor("s1_in", [SPC, N, N], BF16, kind="ExternalInput").ap()
    s2_dram = nc.dram_tensor("s2_in", [SPC, N, N], F8, kind="ExternalInput").ap()
    g64_dram = nc.dram_tensor("g64_in", [64, 64], F32, kind="ExternalInput").ap()
    id64_dram = nc.dram_tensor("id64_in", [64, 64], F32, kind="ExternalInput").ap()
    w_dram = nc.dram_tensor("w_out", [SPC, N], F32, kind="ExternalOutput").ap()

    stop_at = os.environ.get("KM_DBG_STOP", "full")

    with tile.TileContext(nc) as tc, ExitStack() as ctx:
        const_pool = ctx.enter_context(tc.tile_pool(name="const", bufs=1))
        sig_pool = ctx.enter_context(tc.tile_pool(name="sig", bufs=1))
        state_pool = ctx.enter_context(tc.tile_pool(name="state", bufs=1))
        stage_pool = ctx.enter_context(tc.tile_pool(name="stage", bufs=1))
        mv_pool = ctx.enter_context(tc.tile_pool(name="mv", bufs=1, space="PSUM"))
        tr_pool = ctx.enter_context(tc.tile_pool(name="tr", bufs=2, space="PSUM"))
        nw_pool = ctx.enter_context(tc.tile_pool(name="nw", bufs=1, space="PSUM"))

        g64_sb = const_pool.tile([64, 64], F32)
        nc.sync.dma_start(out=g64_sb, in_=g64_dram)
        id64_sb = const_pool.tile([64, 64], F32)
        nc.sync.dma_start(out=id64_sb, in_=id64_dram)

        # Resident Sigma: [part p, sample, chunk c, elem] = S[s][128c+p, e]
        s1_sb = sig_pool.tile([128, SPC, 4, N], BF16, tag="s1")
        s2_sb = sig_pool.tile([128, SPC, 4, N], F8, tag="s2")
        for b in range(SPC):
            nc.sync.dma_start(
                out=s1_sb[:, b], in_=s1_dram[b].rearrange("(c p) e -> p c e", p=128)
            )
        for b in range(SPC):
            nc.sync.dma_start(
                out=s2_sb[:, b], in_=s2_dram[b].rearrange("(c p) e -> p c e", p=128)
            )

        # Zero the matvec PSUM banks once so the [0:97] ACT drain copy never
        # reads uninitialized PSUM (only partitions 0,32,64,96 are written).
        mv_ps = []
        for jg in range(4):
            ps = mv_pool.tile([128, N], F32, tag=f"mv{jg}")
            nc.vector.memset(ps, 0.0)
            mv_ps.append(ps)

        class SG:
            pass

        sgs = []
        for sg in range(2):
            s = SG()
            s.idx = sg
            tg = f"g{sg}"
            s0 = sg * SGN
            P = 4 * SGN  # 64 partitions
            s.mu_rep = state_pool.tile([P, 128], F32, tag=f"{tg}_mu")
            # mu A4 layout: partition 4b+q <- mu[s0+b, 128q:128(q+1)]
            nc.sync.dma_start(out=s.mu_rep, in_=mu_dram[s0 : s0 + SGN, :])
            s.invmu = state_pool.tile([P, 128], F32, tag=f"{tg}_imu")
            nc.vector.reciprocal(s.invmu, s.mu_rep)
            s.musq = state_pool.tile([P, 128], F32, tag=f"{tg}_msq")
            nc.vector.tensor_mul(s.musq, s.mu_rep, s.mu_rep)

            s.u = state_pool.tile([P, 128], F32, tag=f"{tg}_u")
            s.r = state_pool.tile([P, 128], F32, tag=f"{tg}_r")
            s.muv = state_pool.tile([P, 128], F32, tag=f"{tg}_muv")
            s.prod = state_pool.tile([P, 128], F32, tag=f"{tg}_prod")
            s.zr = state_pool.tile([P, 128], F32, tag=f"{tg}_zr")
            s.ysf = state_pool.tile([P, 128], F32, tag=f"{tg}_ysf")
            s.pdma = state_pool.tile([P, 128], F32, tag=f"{tg}_pd")
            s.mnstep = state_pool.tile([P, 128], F32, tag=f"{tg}_mns")
            s.xB = state_pool.tile([128, P], BF16, tag=f"{tg}_xB")
            s.xB2 = state_pool.tile([128, P], BF16, tag=f"{tg}_xB2")
            s.ab = state_pool.tile([P, 2], F32, tag=f"{tg}_ab")
            s.nd = state_pool.tile([P, 2], F32, tag=f"{tg}_nd")
            s.neglam = state_pool.tile([P, 1], F32, tag=f"{tg}_nl")
            s.lam = state_pool.tile([P, 1], F32, tag=f"{tg}_lam")
            s.rb = state_pool.tile([P, 1], F32, tag=f"{tg}_rb")
            s.bmax = state_pool.tile([P, 1], F32, tag=f"{tg}_bm")
            s.negstep = state_pool.tile([P, 1], F32, tag=f"{tg}_ns")
            s.invnegstep = state_pool.tile([P, 1], F32, tag=f"{tg}_ins")
            sgs.append(s)

        def matvec(s, dst, late):
            """dst[A4] = matvec of current stationary x_B (+ x_B2 vs S2)."""
            for jg in range(4):
                ps = mv_ps[jg]
                for j in range(4):
                    b = SGN * s.idx + 4 * jg + j
                    col = 4 * (4 * jg + j)
                    for p in range(4):
                        nc.tensor.matmul(
                            ps[32 * j : 32 * j + 1, :],
                            s.xB[:, col + p : col + p + 1],
                            s1_sb[:, b, p, :],
                            start=(p == 0),
                            stop=(p == 3 and not late),
                            tile_position=(0, 32 * j),
                        )
                    if late:
                        for p in range(4):
                            nc.tensor.matmul(
                                ps[32 * j : 32 * j + 1, :],
                                s.xB2[:, col + p : col + p + 1],
                                s2_sb[:, b, p, :],
                                start=False,
                                stop=(p == 3),
                                tile_position=(0, 32 * j),
                            )
                stage = stage_pool.tile([97, N], F32, tag="st")
                nc.scalar.copy(stage, ps[0:97, :])
                # scatter: stage[32j, 128q:128q+128] -> dst[16jg+4j+q, :]
                nc.sync.dma_start(
                    out=dst[16 * jg : 16 * jg + 16, :], in_=stage[0:97:32, :]
                )

        def to_B(s, src_f32, make_xb2):
            trp = tr_pool.tile([128, 64], F32, tag="tr")
            nc.tensor.transpose(trp, src_f32, id64_sb)
            nc.scalar.copy(s.xB, trp)
            if make_xb2:
                nc.scalar.mul(s.xB2, s.xB, 1.0 / S2_SCALE)

        def gmm(s, rhs, out_ps, n):
            nc.tensor.matmul(
                out_ps[:, 0:n], g64_sb, rhs[:, 0:n], start=True, stop=True
            )

        def newton(s, q, wr, k_iters):
            for _ in range(k_iters):
                nc.vector.scalar_tensor_tensor(
                    out=s.prod, in0=q, scalar=s.neglam[:, 0:1], in1=wr,
                    op0=Alu.is_gt, op1=Alu.mult, accum_out=s.ab[:, 0:1],
                )
                nc.vector.scalar_tensor_tensor(
                    out=s.prod, in0=q, scalar=s.neglam[:, 0:1], in1=s.musq,
                    op0=Alu.is_gt, op1=Alu.mult, accum_out=s.ab[:, 1:2],
                )
                abp = nw_pool.tile([64, 2], F32, tag=f"nw{s.idx}")
                gmm(s, s.ab, abp, 2)
                nc.vector.tensor_scalar(
                    out=s.bmax, in0=abp[:, 1:2], scalar1=1e-30, scalar2=None,
                    op0=Alu.max,
                )
                nc.vector.reciprocal(s.rb, s.bmax)
                nc.vector.scalar_tensor_tensor(
                    out=s.neglam, in0=abp[:, 0:1], scalar=-1.0, in1=s.rb,
                    op0=Alu.add, op1=Alu.mult,
                )

        def finish_lam(s):
            nc.vector.tensor_scalar(
                out=s.lam, in0=s.neglam, scalar1=-1.0, scalar2=None, op0=Alu.mult
            )

        # ---- power iteration (unnormalized) + Rayleigh step size ----
        for s in sgs:
            nc.vector.memset(s.xB, 1.0)
        for k in range(POWER_ITERS):
            for s in sgs:
                matvec(s, s.pdma, late=False)
                if k == POWER_ITERS - 1:
                    # keep v for the Rayleigh quotient
                    nc.vector.tensor_copy(s.u, s.pdma)
                to_B(s, s.pdma, make_xb2=False)
        for s in sgs:
            matvec(s, s.muv, late=False)  # w = Sigma v
        for s in sgs:
            nc.vector.scalar_tensor_tensor(
                out=s.prod, in0=s.u, scalar=0.0, in1=s.muv,
                op0=Alu.add, op1=Alu.mult, accum_out=s.nd[:, 0:1],
            )
            nc.vector.scalar_tensor_tensor(
                out=s.prod, in0=s.u, scalar=0.0, in1=s.u,
                op0=Alu.add, op1=Alu.mult, accum_out=s.nd[:, 1:2],
            )
            nwp = nw_pool.tile([64, 2], F32, tag=f"nw{s.idx}")
            gmm(s, s.nd, nwp, 2)
            # negstep = -(v.v)/(v.w) = -1/lmax ; invnegstep = -(v.w)/(v.v) = -lmax
            nc.vector.reciprocal(s.rb, nwp[:, 0:1])
            nc.vector.scalar_tensor_tensor(
                out=s.negstep, in0=nwp[:, 1:2], scalar=-1.0, in1=s.rb,
                op0=Alu.mult, op1=Alu.mult,
            )
            nc.vector.reciprocal(s.bmax, nwp[:, 1:2])
            nc.vector.scalar_tensor_tensor(
                out=s.invnegstep, in0=nwp[:, 0:1], scalar=-1.0, in1=s.bmax,
                op0=Alu.mult, op1=Alu.mult,
            )
            nc.vector.tensor_scalar(
                out=s.mnstep, in0=s.mu_rep, scalar1=s.negstep[:, 0:1],
                scalar2=None, op0=Alu.mult,
            )

        if stop_at == "power":
            for s in sgs:
                nc.sync.dma_start(
                    out=w_dram[s.idx * SGN : (s.idx + 1) * SGN, :], in_=s.u
                )
            nc.compile()
            return nc

        # ---- y0 = project(ones): u=ones -> r=invmu, muv=mu ----
        for s in sgs:
            nc.vector.memset(s.neglam, -1e30)
            newton(s, s.invmu, s.mu_rep, NEWTON_K0)
            finish_lam(s)
            nc.scalar.activation(s.zr, s.invmu, Act.Relu, bias=s.lam[:, 0:1])
            nc.vector.tensor_mul(s.ysf, s.mnstep, s.zr)
            to_B(s, s.ysf, make_xb2=(SPLIT_FROM == 0))

        if stop_at == "y0":
            for s in sgs:
                nc.sync.dma_start(
                    out=w_dram[s.idx * SGN : (s.idx + 1) * SGN, :], in_=s.ysf
                )
            nc.compile()
            return nc

        # ---- PGD ----
        for k in range(PGD_ITERS):
            late = k >= SPLIT_FROM
            last = k == PGD_ITERS - 1
            for s in sgs:
                matvec(s, s.pdma, late=late)
            for s in sgs:
                # u = y - step*Sigma y = ysf*(-lmax) + pdma
                nc.vector.scalar_tensor_tensor(
                    out=s.u, in0=s.ysf, scalar=s.invnegstep[:, 0:1], in1=s.pdma,
                    op0=Alu.mult, op1=Alu.add,
                )
                nc.vector.tensor_mul(s.r, s.u, s.invmu)
                nc.vector.tensor_mul(s.muv, s.u, s.mu_rep)
                newton(s, s.r, s.muv, NEWTON_K)
                finish_lam(s)
                nc.scalar.activation(s.zr, s.r, Act.Relu, bias=s.lam[:, 0:1])
                nc.vector.tensor_mul(s.ysf, s.mnstep, s.zr)
                if not last:
                    to_B(s, s.ysf, make_xb2=(k + 1 >= SPLIT_FROM - 1))

        # ---- postprocess: w = ysf / sum(ysf)  (scale/sign cancel) ----
        for s in sgs:
            sp = s.nd
            nc.vector.tensor_scalar(
                out=s.prod, in0=s.ysf, scalar1=1.0, scalar2=None,
                op0=Alu.mult, op1=Alu.add, accum_out=sp[:, 0:1],
            )
            spp = nw_pool.tile([64, 2], F32, tag=f"nw{s.idx}")
            gmm(s, sp, spp, 1)
            nc.vector.reciprocal(s.rb, spp[:, 0:1])
            wf = s.u
            nc.vector.tensor_scalar(
                out=wf, in0=s.ysf, scalar1=s.rb[:, 0:1], scalar2=None, op0=Alu.mult
            )
            nc.sync.dma_start(
                out=w_dram[s.idx * SGN : (s.idx + 1) * SGN, :], in_=wf
            )

    nc.compile()
    return nc


def _get_program():
    if "nc" not in _PROGRAM_CACHE:
        _PROGRAM_CACHE["nc"] = _build_program()
    return _PROGRAM_CACHE["nc"]


def _host_inputs(mu, sig):
    """Per-core input maps: precision-split Sigma + tiny constants."""
    import ml_dtypes

    s1 = sig.astype(ml_dtypes.bfloat16)
    s2 = ((sig - s1.astype(np.float32)) * S2_SCALE).astype(ml_dtypes.float8_e4m3fn)
    g64 = np.kron(np.eye(SGN, dtype=np.float32), np.ones((4, 4), np.float32))
    id64 = np.eye(64, dtype=np.float32)
    in_maps = []
    for c in range(NCORES):
        sl = slice(c * SPC, (c + 1) * SPC)
        in_maps.append(
            {
                "mu_in": np.ascontiguousarray(mu[sl]),
                "s1_in": np.ascontiguousarray(s1[sl]),
                "s2_in": np.ascontiguousarray(s2[sl]),
                "g64_in": g64,
                "id64_in": id64,
            }
        )
    return in_maps


def kernel(predicted_returns: np.ndarray, covariance_matrix: np.ndarray) -> np.ndarray:
    from concourse.bass_utils import run_bass_kernel_spmd

    mu = np.ascontiguousarray(predicted_returns, dtype=np.float32)
    sig = np.ascontiguousarray(covariance_matrix, dtype=np.float32)
    batch = mu.shape[0]
    assert batch == NCORES * SPC and mu.shape[1] == N

    nc = _get_program()
    in_maps = _host_inputs(mu, sig)
    res = run_bass_kernel_spmd(nc, in_maps, core_ids=list(range(NCORES)))
    out = np.concatenate([r["w_out"] for r in res.results], axis=0)
    return out.astype(np.float32)


if __name__ == "__main__":
    rng = np.random.default_rng(0)
    mu = (0.05 + 0.1 * rng.random((NCORES * SPC, N))).astype(np.float32)
    A = rng.standard_normal((4, N, N)).astype(np.float32)
    sig = np.einsum("bik,bjk->bij", A, A) / N + 0.1 * np.eye(N, dtype=np.float32)
    sig = np.tile(sig, (64, 1, 1)).astype(np.float32)
    w = kernel(mu, sig)
    print(w.shape, w.sum(axis=1)[:4])


# revision 9
# speedup vs baseline: 1.2956x; 1.0450x over previous
"""Trainium2 Bass kernel for batched differentiable mean-variance optimization.

Problem: for each of 256 samples, solve
    min 0.5 y^T Sigma y  s.t.  mu^T y = 1, y >= 0
via 150 unrolled projected-gradient iterations (step = 1/lambda_max via power
iteration), then normalize to portfolio weights.  Pure data parallel across 8
cores (32 samples/core).

Design (v2):
- Sigma is shipped from the host as a precision pair: S1 = bf16(Sigma)
  (16 MB/core) plus S2 = fp8_e4m3((Sigma - S1) * 4096) (8 MB/core).  Both stay
  SBUF-resident for all 32 samples.
- Matvec Sigma @ y streams S1 chunks as the PE moving operand at 1 col/cycle
  with 4 samples running concurrently in distinct 32-column groups
  (tile_position), i.e. 512 cycles/sample/matvec.  Error-decay analysis shows
  bf16 matvec noise injected at PGD iter k fades by ~0.976^(150-k), so only
  the last 40 iterations add the fp8-scaled S2 correction matmuls
  (compensated product, ~fp32-quality fixed point) at 2x PE cost.
- Power iteration runs 10 unnormalized steps + Rayleigh quotient (step size
  only needs ~1e-3 accuracy).
- Projection onto {y>=0, mu@y=1}: warm-started Newton/active-set iterations
  (K=3/iter) in a dense A4 layout [64, 128]: partition = 4*sample+quarter.
  Masked sums fuse into scalar_tensor_tensor with accum_out; the 4-partition
  per-sample reduce+broadcast is one tiny PE matmul against a block G matrix.
  ScalarE (ACT) takes the PSUM drains, the relu projection, and the
  PSUM->SBUF transpose copies so DVE keeps only the tensor_tensor work.
- Two subgroups of 16 samples self-pipeline via the Tile list scheduler
  (PE matvec of one subgroup overlaps DVE projection of the other).
- Final weights w = y/sum(y) are computed from ys = -step*mu*z directly
  (per-sample positive rescale cancels in the normalization; relu/renorm
  of the reference are no-ops here since w >= 0 elementwise).
"""

import os
import numpy as np
from contextlib import ExitStack

N = 512
NCORES = 8
SPC = 32            # samples per core (all SBUF-resident)
SGN = 16            # samples per subgroup
POWER_ITERS = 10
PGD_ITERS = 150
SPLIT_FROM = 110    # first PGD iter that applies the S2 correction
NEWTON_K = 3        # warm-started Newton iters per projection
NEWTON_K0 = 8       # cold-start Newton iters for y0
S2_SCALE = 4096.0

_PROGRAM_CACHE = {}


def _build_program(pgd_iters=PGD_ITERS, split_from=SPLIT_FROM,
                   power_iters=POWER_ITERS, newton_k=NEWTON_K):
    import concourse.bacc as bacc
    import concourse.tile as tile
    from concourse import mybir

    Alu = mybir.AluOpType
    Act = mybir.ActivationFunctionType
    F32 = mybir.dt.float32
    BF16 = mybir.dt.bfloat16
    F8 = mybir.dt.float8e4

    nc = bacc.Bacc(
        "TRN2",
        target_bir_lowering=False,
        debug=False,
        enable_asserts=False,
        num_devices=NCORES,
    )

    mu_dram = nc.dram_tensor("mu_in", [SPC, N], F32, kind="ExternalInput").ap()
    s1_dram = nc.dram_tensor("s1_in", [SPC, N, N], BF16, kind="ExternalInput").ap()
    s2_dram = nc.dram_tensor("s2_in", [SPC, N, N], F8, kind="ExternalInput").ap()
    g64_dram = nc.dram_tensor("g64_in", [64, 64], F32, kind="ExternalInput").ap()
    id64_dram = nc.dram_tensor("id64_in", [64, 64], F32, kind="ExternalInput").ap()
    w_dram = nc.dram_tensor("w_out", [SPC, N], F32, kind="ExternalOutput").ap()

    stop_at = os.environ.get("KM_DBG_STOP", "full")

    with tile.TileContext(nc) as tc, ExitStack() as ctx:
        const_pool = ctx.enter_context(tc.tile_pool(name="const", bufs=1))
        sig_pool = ctx.enter_context(tc.tile_pool(name="sig", bufs=1))
        state_pool = ctx.enter_context(tc.tile_pool(name="state", bufs=1))
        stage_pool = ctx.enter_context(tc.tile_pool(name="stage", bufs=1))
        mv_pool = ctx.enter_context(tc.tile_pool(name="mv", bufs=1, space="PSUM"))
        tr_pool = ctx.enter_context(tc.tile_pool(name="tr", bufs=2, space="PSUM"))
        nw_pool = ctx.enter_context(tc.tile_pool(name="nw", bufs=1, space="PSUM"))

        g64_sb = const_pool.tile([64, 64], F32)
        nc.sync.dma_start(out=g64_sb, in_=g64_dram)
        id64_sb = const_pool.tile([64, 64], F32)
        nc.sync.dma_start(out=id64_sb, in_=id64_dram)

        # Resident Sigma: [part p, sample, chunk c, elem] = S[s][128c+p, e]
        s1_sb = sig_pool.tile([128, SPC, 4, N], BF16, tag="s1")
        s2_sb = sig_pool.tile([128, SPC, 4, N], F8, tag="s2")
        for b in range(SPC):
            nc.sync.dma_start(
                out=s1_sb[:, b], in_=s1_dram[b].rearrange("(c p) e -> p c e", p=128)
            )
        for b in range(SPC):
            nc.sync.dma_start(
                out=s2_sb[:, b], in_=s2_dram[b].rearrange("(c p) e -> p c e", p=128)
            )

        # Zero the matvec PSUM banks once so the [0:97] ACT drain copy never
        # reads uninitialized PSUM (only partitions 0,32,64,96 are written).
        mv_ps = []
        for jg in range(4):
            ps = mv_pool.tile([128, N], F32, tag=f"mv{jg}")
            nc.vector.memset(ps, 0.0)
            mv_ps.append(ps)

        class SG:
            pass

        sgs = []
        for sg in range(2):
            s = SG()
            s.idx = sg
            tg = f"g{sg}"
            s0 = sg * SGN
            P = 4 * SGN  # 64 partitions
            s.mu_rep = state_pool.tile([P, 128], F32, tag=f"{tg}_mu")
            # mu A4 layout: partition 4b+q <- mu[s0+b, 128q:128(q+1)]
            nc.sync.dma_start(out=s.mu_rep, in_=mu_dram[s0 : s0 + SGN, :])
            s.invmu = state_pool.tile([P, 128], F32, tag=f"{tg}_imu")
            nc.vector.reciprocal(s.invmu, s.mu_rep)
            s.musq = state_pool.tile([P, 128], F32, tag=f"{tg}_msq")
            nc.vector.tensor_mul(s.musq, s.mu_rep, s.mu_rep)

            s.u = state_pool.tile([P, 128], F32, tag=f"{tg}_u")
            s.r = state_pool.tile([P, 128], F32, tag=f"{tg}_r")
            s.muv = state_pool.tile([P, 128], F32, tag=f"{tg}_muv")
            s.zr = s.muv  # relu output reuses muv (dead after newton)
            s.ysf = state_pool.tile([P, 128], F32, tag=f"{tg}_ysf")
            s.pdma = state_pool.tile([P, 128], F32, tag=f"{tg}_pd")
            s.xB = state_pool.tile([128, P], BF16, tag=f"{tg}_xB")
            s.xB2 = state_pool.tile([128, P], BF16, tag=f"{tg}_xB2")
            s.ab = state_pool.tile([P, 2], F32, tag=f"{tg}_ab")
            s.nd = state_pool.tile([P, 2], F32, tag=f"{tg}_nd")
            s.neglam = state_pool.tile([P, 1], F32, tag=f"{tg}_nl")
            s.lam = state_pool.tile([P, 1], F32, tag=f"{tg}_lam")
            s.rb = state_pool.tile([P, 1], F32, tag=f"{tg}_rb")
            s.bmax = state_pool.tile([P, 1], F32, tag=f"{tg}_bm")
            s.negstep = state_pool.tile([P, 1], F32, tag=f"{tg}_ns")
            s.invnegstep = state_pool.tile([P, 1], F32, tag=f"{tg}_ins")
            sgs.append(s)

        def matvec(s, dst, late):
            """dst[A4] = matvec of current stationary x_B (+ x_B2 vs S2)."""
            for jg in range(4):
                ps = mv_ps[jg]
                # p-outer, j-inner: 4 independent col-group streams issue
                # back-to-back and run concurrently; each sample's p-chain
                # advances once per round (same-group matmuls serialize).
                for p in range(4):
                    for j in range(4):
                        b = SGN * s.idx + 4 * jg + j
                        col = 4 * (4 * jg + j)
                        nc.tensor.matmul(
                            ps[32 * j : 32 * j + 1, :],
                            s.xB[:, col + p : col + p + 1],
                            s1_sb[:, b, p, :],
                            start=(p == 0),
                            stop=(p == 3 and not late),
                            tile_position=(0, 32 * j),
                        )
                if late:
                    for p in range(4):
                        for j in range(4):
                            b = SGN * s.idx + 4 * jg + j
                            col = 4 * (4 * jg + j)
                            nc.tensor.matmul(
                                ps[32 * j : 32 * j + 1, :],
                                s.xB2[:, col + p : col + p + 1],
                                s2_sb[:, b, p, :],
                                start=False,
                                stop=(p == 3),
                                tile_position=(0, 32 * j),
                            )
                # Full-bank drain through alternating stage buffers so
                # consecutive bank drains pipeline (copy of jg+1 overlaps
                # the scatter DMA of jg).
                stage = stage_pool.tile([97, N], F32, tag=f"st{jg % 2}")
                nc.scalar.copy(stage, ps[0:97, :])
                # stage[32j, 128q+f] -> dst[16jg+4j+q, f]
                nc.sync.dma_start(
                    out=dst[16 * jg : 16 * jg + 16, :],
                    in_=stage[0:97:32, :],
                )

        def to_B(s, src_f32, make_xb2):
            trp = tr_pool.tile([128, 64], F32, tag="tr")
            nc.tensor.transpose(trp, src_f32, id64_sb)
            nc.scalar.copy(s.xB, trp)
            if make_xb2:
                nc.scalar.mul(s.xB2, s.xB, 1.0 / S2_SCALE)

        def gmm(s, rhs, out_ps, n):
            nc.tensor.matmul(
                out_ps[:, 0:n], g64_sb, rhs[:, 0:n], start=True, stop=True
            )

        def newton(s, q, wr, k_iters, scratch):
            for _ in range(k_iters):
                nc.vector.scalar_tensor_tensor(
                    out=scratch, in0=q, scalar=s.neglam[:, 0:1], in1=wr,
                    op0=Alu.is_gt, op1=Alu.mult, accum_out=s.ab[:, 0:1],
                )
                nc.vector.scalar_tensor_tensor(
                    out=scratch, in0=q, scalar=s.neglam[:, 0:1], in1=s.musq,
                    op0=Alu.is_gt, op1=Alu.mult, accum_out=s.ab[:, 1:2],
                )
                abp = nw_pool.tile([64, 2], F32, tag=f"nw{s.idx}")
                gmm(s, s.ab, abp, 2)
                nc.vector.tensor_scalar(
                    out=s.bmax, in0=abp[:, 1:2], scalar1=1e-30, scalar2=None,
                    op0=Alu.max,
                )
                nc.vector.reciprocal(s.rb, s.bmax)
                nc.vector.scalar_tensor_tensor(
                    out=s.neglam, in0=abp[:, 0:1], scalar=-1.0, in1=s.rb,
                    op0=Alu.add, op1=Alu.mult,
                )

        def finish_lam(s):
            nc.vector.tensor_scalar(
                out=s.lam, in0=s.neglam, scalar1=-1.0, scalar2=None, op0=Alu.mult
            )

        # ---- power iteration (unnormalized) + Rayleigh step size ----
        for s in sgs:
            nc.vector.memset(s.xB, 1.0)
        for k in range(power_iters):
            for s in sgs:
                matvec(s, s.pdma, late=False)
                if k == power_iters - 1:
                    # keep v for the Rayleigh quotient
                    nc.vector.tensor_copy(s.u, s.pdma)
                to_B(s, s.pdma, make_xb2=False)
        for s in sgs:
            matvec(s, s.muv, late=False)  # w = Sigma v
        for s in sgs:
            nc.vector.scalar_tensor_tensor(
                out=s.r, in0=s.u, scalar=0.0, in1=s.muv,
                op0=Alu.add, op1=Alu.mult, accum_out=s.nd[:, 0:1],
            )
            nc.vector.scalar_tensor_tensor(
                out=s.r, in0=s.u, scalar=0.0, in1=s.u,
                op0=Alu.add, op1=Alu.mult, accum_out=s.nd[:, 1:2],
            )
            nwp = nw_pool.tile([64, 2], F32, tag=f"nw{s.idx}")
            gmm(s, s.nd, nwp, 2)
            # negstep = -(v.v)/(v.w) = -1/lmax ; invnegstep = -(v.w)/(v.v) = -lmax
            nc.vector.reciprocal(s.rb, nwp[:, 0:1])
            nc.vector.scalar_tensor_tensor(
                out=s.negstep, in0=nwp[:, 1:2], scalar=-1.0, in1=s.rb,
                op0=Alu.mult, op1=Alu.mult,
            )
            nc.vector.reciprocal(s.bmax, nwp[:, 1:2])
            nc.vector.scalar_tensor_tensor(
                out=s.invnegstep, in0=nwp[:, 0:1], scalar=-1.0, in1=s.bmax,
                op0=Alu.mult, op1=Alu.mult,
            )

        if stop_at == "power":
            for s in sgs:
                nc.sync.dma_start(
                    out=w_dram[s.idx * SGN : (s.idx + 1) * SGN, :], in_=s.u
                )
            nc.compile()
            return nc

        # ---- y0 = project(ones): u=ones -> r=invmu, muv=mu ----
        for s in sgs:
            nc.vector.memset(s.neglam, -1e30)
            newton(s, s.invmu, s.mu_rep, NEWTON_K0, s.pdma)
            finish_lam(s)
            nc.scalar.activation(s.zr, s.invmu, Act.Relu, bias=s.lam[:, 0:1])
            nc.vector.scalar_tensor_tensor(
                out=s.ysf, in0=s.mu_rep, scalar=s.negstep[:, 0:1], in1=s.zr,
                op0=Alu.mult, op1=Alu.mult,
            )
            to_B(s, s.ysf, make_xb2=(split_from == 0))

        if stop_at == "y0":
            for s in sgs:
                nc.sync.dma_start(
                    out=w_dram[s.idx * SGN : (s.idx + 1) * SGN, :], in_=s.ysf
                )
            nc.compile()
            return nc

        # ---- PGD ----
        for k in range(pgd_iters):
            late = k >= split_from
            last = k == pgd_iters - 1
            for s in sgs:
                matvec(s, s.pdma, late=late)
            for s in sgs:
                # u = y - step*Sigma y = ysf*(-lmax) + pdma
                nc.vector.scalar_tensor_tensor(
                    out=s.u, in0=s.ysf, scalar=s.invnegstep[:, 0:1], in1=s.pdma,
                    op0=Alu.mult, op1=Alu.add,
                )
                nc.vector.tensor_mul(s.r, s.u, s.invmu)
                nc.vector.tensor_mul(s.muv, s.u, s.mu_rep)
                newton(s, s.r, s.muv, newton_k, s.pdma)
                finish_lam(s)
                nc.scalar.activation(s.zr, s.r, Act.Relu, bias=s.lam[:, 0:1])
                nc.vector.scalar_tensor_tensor(
                    out=s.ysf, in0=s.mu_rep, scalar=s.negstep[:, 0:1], in1=s.zr,
                    op0=Alu.mult, op1=Alu.mult,
                )
                if not last:
                    to_B(s, s.ysf, make_xb2=(k + 1 >= split_from - 1))

        # ---- postprocess: w = ysf / sum(ysf)  (scale/sign cancel) ----
        for s in sgs:
            sp = s.nd
            nc.vector.tensor_scalar(
                out=s.r, in0=s.ysf, scalar1=1.0, scalar2=None,
                op0=Alu.mult, op1=Alu.add, accum_out=sp[:, 0:1],
            )
            spp = nw_pool.tile([64, 2], F32, tag=f"nw{s.idx}")
            gmm(s, sp, spp, 1)
            nc.vector.reciprocal(s.rb, spp[:, 0:1])
            wf = s.u
            nc.vector.tensor_scalar(
                out=wf, in0=s.ysf, scalar1=s.rb[:, 0:1], scalar2=None, op0=Alu.mult
            )
            nc.sync.dma_start(
                out=w_dram[s.idx * SGN : (s.idx + 1) * SGN, :], in_=wf
            )

    nc.compile()
    return nc


def _get_program():
    if "nc" not in _PROGRAM_CACHE:
        _PROGRAM_CACHE["nc"] = _build_program()
    return _PROGRAM_CACHE["nc"]


def _host_inputs(mu, sig):
    """Per-core input maps: precision-split Sigma + tiny constants."""
    import ml_dtypes

    s1 = sig.astype(ml_dtypes.bfloat16)
    s2 = ((sig - s1.astype(np.float32)) * S2_SCALE).astype(ml_dtypes.float8_e4m3fn)
    g64 = np.kron(np.eye(SGN, dtype=np.float32), np.ones((4, 4), np.float32))
    id64 = np.eye(64, dtype=np.float32)
    in_maps = []
    for c in range(NCORES):
        sl = slice(c * SPC, (c + 1) * SPC)
        in_maps.append(
            {
                "mu_in": np.ascontiguousarray(mu[sl]),
                "s1_in": np.ascontiguousarray(s1[sl]),
                "s2_in": np.ascontiguousarray(s2[sl]),
                "g64_in": g64,
                "id64_in": id64,
            }
        )
    return in_maps


def kernel(predicted_returns: np.ndarray, covariance_matrix: np.ndarray) -> np.ndarray:
    from concourse.bass_utils import run_bass_kernel_spmd

    mu = np.ascontiguousarray(predicted_returns, dtype=np.float32)
    sig = np.ascontiguousarray(covariance_matrix, dtype=np.float32)
    batch = mu.shape[0]
    assert batch == NCORES * SPC and mu.shape[1] == N

    nc = _get_program()
    in_maps = _host_inputs(mu, sig)
    res = run_bass_kernel_spmd(nc, in_maps, core_ids=list(range(NCORES)))
    out = np.concatenate([r["w_out"] for r in res.results], axis=0)
    return out.astype(np.float32)


if __name__ == "__main__":
    rng = np.random.default_rng(0)
    mu = (0.05 + 0.1 * rng.random((NCORES * SPC, N))).astype(np.float32)
    A = rng.standard_normal((4, N, N)).astype(np.float32)
    sig = np.einsum("bik,bjk->bij", A, A) / N + 0.1 * np.eye(N, dtype=np.float32)
    sig = np.tile(sig, (64, 1, 1)).astype(np.float32)
    w = kernel(mu, sig)
    print(w.shape, w.sum(axis=1)[:4])
